# revision 1
# baseline (speedup 1.0000x reference)
# Trainium2 Bass kernel for nn_CompressedGPT2Attention.
#
# Model: B=2, S=2048, D=1024, H=16 heads of HD=64.
#   qkv = x @ c_attn_w + c_attn_b ; causal attention per head;
#   per-head symmetric projector on the attention output; out = attn @ c_proj_w + b.
#
# Sharding (megatron-style tensor parallel over heads, 8 cores x 2 heads):
#   - every core gets the full hidden_states
#   - c_attn (q,k,v) columns + projectors + c_proj rows are sharded by head
#   - each core computes a full-shape partial of the c_proj output; the
#     all-reduce after c_proj is done on the host (partials are summed there).
#
# On-core layout strategy: activations are kept feature-major ("transposed",
# features on SBUF partitions) so every matmul contracts over the partition
# dim without ever transposing big intermediates:
#   xT[d, s]   provided by the host (input marshalling) and cast to bf16
#              by the SWDGE load DMA
#   qT,kT[f,s] = W^T @ xT
#   scoresT[kj, qi] = kT^T-slice matmuls (two heads packed on the PE via
#                     tile_position row-tiling, K=64 each)
#   expT = exp(scoresT/8) on ScalarE, causal mask via gpsimd affine_select
#   v[s, hd]   computed directly in sequence-major layout (xT stationary)
#   attn_unT[hd, qi] accumulated over kj with lhsT = v; softmax sums
#                     ride along as a concurrent ones-column matmul
#   attnP_T[e, qi]  = projector matmul (two heads packed, K=64)
#   normalization   = (1/sums) broadcast across partitions with a K=1 matmul,
#                     then one VectorE multiply
#   outT[dout, s]   = c_proj partial, written back fp32; host sums over cores.

import numpy as np

B, S, D, H, HD = 2, 2048, 1024, 16, 64
BS = B * S
N_CORES = 8
HPC = H // N_CORES  # heads per core = 2

_CACHE = {}
USE_CRIT = False  # tile_critical around paired matmuls faults the device


def _build(nc):
    import concourse.bass as bass
    import concourse.mybir as mybir
    import concourse.tile as tile
    from contextlib import ExitStack

    f32 = mybir.dt.float32
    bf16 = mybir.dt.bfloat16
    AF = mybir.ActivationFunctionType
    OP = mybir.AluOpType

    x_d = nc.dram_tensor("xT", [D, BS], bf16, kind="ExternalInput").ap()
    wqk_d = nc.dram_tensor("w_qk", [D, 2 * HPC * HD], bf16, kind="ExternalInput").ap()
    wv_d = nc.dram_tensor("w_v", [D, HPC * HD], bf16, kind="ExternalInput").ap()
    bqk_d = nc.dram_tensor("b_qk", [2 * HPC * HD], f32, kind="ExternalInput").ap()
    bv_d = nc.dram_tensor("b_v", [HPC * HD], f32, kind="ExternalInput").ap()
    wpr_d = nc.dram_tensor("w_pr", [HPC * HD, HD], bf16, kind="ExternalInput").ap()
    wcp_d = nc.dram_tensor("w_cp", [HPC * HD, D], bf16, kind="ExternalInput").ap()
    bcp_d = nc.dram_tensor("b_cp", [D], f32, kind="ExternalInput").ap()
    out_d = nc.dram_tensor("outT", [8, 128, BS], f32, kind="ExternalOutput").ap()

    F = HPC * HD  # 128 features per block (2 heads stacked)
    NB = BS // 512  # 8 s-blocks of 512
    KT = D // 128  # 8 contraction tiles

    from contextlib import nullcontext

    with TileCtx(tile, nc) as tc:
        crit = (lambda: tc.tile_critical()) if USE_CRIT else (lambda: nullcontext())
        # ---------------- persistent tiles ----------------
        # tc.tile singles must be released in LIFO order, and their free
        # closures must be kept alive (GC of a discarded closure releases
        # the pool at a random trace point). xT goes last so it can be
        # freed right after the qkv phase.
        frees = []

        def ptile(shape, dtype, name):
            t, free = tc.tile(shape, dtype, name=name)
            frees.append(free)
            return t

        qT = ptile([128, BS], bf16, "qT")
        kTt = ptile([128, BS], bf16, "kTt")
        v_s = ptile([128, BS // 128, 128], bf16, "v_s")
        wqk_sb = ptile([128, KT, 2 * F], bf16, "wqk_sb")
        wv_sb = ptile([128, KT, F], bf16, "wv_sb")
        wpr_sb = ptile([128, HD], bf16, "wpr_sb")
        wcp_sb = ptile([128, D], bf16, "wcp_sb")
        bqk_sb = ptile([128, 2], f32, "bqk_sb")
        bcp_sb = ptile([128, 8], f32, "bcp_sb")
        ones_w = ptile([128, 64], bf16, "ones_w")
        ones_row = ptile([1, 128], bf16, "ones_row")
        bv16 = ptile([1, 128], bf16, "bv16")
        bias_v_bc = ptile([128, 128], f32, "bias_v_bc")
        # one tile per 512-wide s-block so c_proj can start per-block
        cpr = [ptile([128, 512], bf16, f"cpr{i}") for i in range(NB)]

        xT, xT_free = tc.tile([128, KT, BS], bf16, name="xT")

        # ---------------- constants + weights ----------------
        nc.any.memset(ones_w[:], 1.0)
        nc.any.memset(ones_row[:], 1.0)
        nc.sync.dma_start(wqk_sb[:], wqk_d.rearrange("(kt p) f -> p kt f", p=128))
        nc.sync.dma_start(wv_sb[:], wv_d.rearrange("(kt p) f -> p kt f", p=128))
        nc.sync.dma_start(wpr_sb[:], wpr_d)
        nc.sync.dma_start(wcp_sb[:], wcp_d)
        nc.sync.dma_start(bqk_sb[:], bqk_d.rearrange("(t p) -> p t", p=128))
        nc.gpsimd.dma_start(bv16[:], bv_d[None, :])
        nc.sync.dma_start(bcp_sb[:], bcp_d.rearrange("(t p) -> p t", p=128))

        # ---------------- load x^T (bf16, marshalled on the host) -------------
        with ExitStack() as phase1:
            for kt in range(KT):
                nc.sync.dma_start(
                    xT[:, kt, :], x_d[kt * 128 : (kt + 1) * 128, :]
                )

            qkv_ps = phase1.enter_context(
                tc.tile_pool(name="qkv_ps", bufs=3, space="PSUM")
            )
            vt_ps = phase1.enter_context(
                tc.tile_pool(name="vt_ps", bufs=1, space="PSUM")
            )

            # ---------------- q^T / k^T matmuls ----------------
            for ft in range(2):  # 0=q, 1=k
                dest = (qT, kTt)[ft]
                for sb in range(NB):
                    ps = qkv_ps.tile([128, 512], f32, tag="qkv")
                    for kt in range(KT):
                        nc.tensor.matmul(
                            ps[:],
                            wqk_sb[:, kt, ft * F : (ft + 1) * F],
                            xT[:, kt, sb * 512 : (sb + 1) * 512],
                            start=(kt == 0),
                            stop=(kt == KT - 1),
                        )
                    nc.scalar.activation(
                        dest[:, sb * 512 : (sb + 1) * 512], ps[:], AF.Identity,
                        bias=bqk_sb[:, ft : ft + 1],
                    )

            # ---------------- v bias broadcast tile ----------------
            # v is produced sequence-major, so its per-feature bias lives on
            # the free dim; build [128,128] tile with every row = b_v once
            # (K=1 matmul against ones) and add it during the psum drain.
            ps_bv = vt_ps.tile([128, 128], f32, tag="vt")
            nc.tensor.matmul(
                ps_bv[:], ones_row[:], bv16[:],
                start=True, stop=True,
            )
            nc.vector.tensor_copy(bias_v_bc[:], ps_bv[:])

            # ---------------- v, directly in sequence-major [s, hd] ----------
            for st in range(BS // 128):
                ps_v = qkv_ps.tile([128, 128], f32, tag="v")
                for kt in range(KT):
                    nc.tensor.matmul(
                        ps_v[:],
                        xT[:, kt, st * 128 : (st + 1) * 128],
                        wv_sb[:, kt, :],
                        start=(kt == 0),
                        stop=(kt == KT - 1),
                    )
                nc.vector.scalar_tensor_tensor(
                    v_s[:, st, :], ps_v[:], 1.0, bias_v_bc[:],
                    OP.mult, OP.add,
                )
        xT_free()

        # ---------------- attention ----------------
        with ExitStack() as phase2:
            sc_ps = phase2.enter_context(tc.tile_pool(name="sc_ps", bufs=4, space="PSUM"))
            attn_ps = phase2.enter_context(tc.tile_pool(name="attn_ps", bufs=2, space="PSUM"))
            aux_ps = phase2.enter_context(tc.tile_pool(name="aux_ps", bufs=2, space="PSUM"))
            epool = phase2.enter_context(tc.tile_pool(name="epool", bufs=6))
            spool = phase2.enter_context(tc.tile_pool(name="spool", bufs=2))
            opool = phase2.enter_context(tc.tile_pool(name="opool", bufs=4))

            for qt in range(4):
                for b in range(B):
                    blk = b * 4 + qt
                    qi = b * S + qt * 512
                    nkj = 4 * (qt + 1)
                    ps_attn = attn_ps.tile([128, 512], f32, tag="attn")
                    ps_sums = aux_ps.tile([128, 512], f32, tag="aux")
                    for kj in range(nkj):
                        kjc = b * S + kj * 128
                        p = kj - 4 * qt
                        pscA = sc_ps.tile([128, 512], f32, tag="sc")
                        pscB = sc_ps.tile([128, 512], f32, tag="sc")
                        nc.tensor.matmul(
                            pscA[:], kTt[0:64, kjc : kjc + 128],
                            qT[0:64, qi : qi + 512],
                            start=True, stop=True, tile_position=(0, 0),
                        )
                        nc.tensor.matmul(
                            pscB[:], kTt[64:128, kjc : kjc + 128],
                            qT[64:128, qi : qi + 512],
                            start=True, stop=True, tile_position=(64, 0),
                        )
                        eA = epool.tile([128, 512], bf16, tag="e")
                        eB = epool.tile([128, 512], bf16, tag="e")
                        for e, psc in ((eA, pscA), (eB, pscB)):
                            if p > 0:
                                # fully-masked left region: zero instead of exp
                                nc.gpsimd.memset(e[:, 0 : 128 * p], 0.0)
                                nc.scalar.activation(
                                    e[:, 128 * p : 512], psc[:, 128 * p : 512],
                                    AF.Exp, scale=0.125,
                                )
                            else:
                                nc.scalar.activation(e[:], psc[:], AF.Exp, scale=0.125)
                            if p >= 0:
                                # triangle mask on the 128-wide diagonal square
                                nc.gpsimd.affine_select(
                                    e[:, 128 * p : 128 * (p + 1)],
                                    e[:, 128 * p : 128 * (p + 1)],
                                    pattern=[[1, 128]], base=0,
                                    channel_multiplier=-1,
                                    compare_op=OP.is_ge, fill=0.0,
                                )
                        first, last = kj == 0, kj == nkj - 1
                        vs = v_s[:, b * 16 + kj, :]
                        nc.tensor.matmul(
                            ps_attn[0:64, :], vs[:, 0:64], eA[:],
                            start=first, stop=last, tile_position=(0, 0),
                            skip_group_check=True,
                        )
                        nc.tensor.matmul(
                            ps_attn[64:128, :], vs[:, 64:128], eB[:],
                            start=first, stop=last, tile_position=(0, 64),
                            skip_group_check=True,
                        )
                        nc.tensor.matmul(
                            ps_sums[0:64, :], ones_w[:, 0:64], eA[:],
                            start=first, stop=last, tile_position=(0, 0),
                            skip_group_check=True,
                        )
                        nc.tensor.matmul(
                            ps_sums[64:128, :], ones_w[:, 0:64], eB[:],
                            start=first, stop=last, tile_position=(0, 64),
                            skip_group_check=True,
                        )

                    attn_sb = spool.tile([128, 512], bf16, tag="attn_sb")
                    nc.vector.tensor_copy(attn_sb[:], ps_attn[:])
                    # sums are matmul-broadcast across partitions, so one
                    # DVE reciprocal yields the normalization tile directly
                    rec_bc = spool.tile([128, 512], f32, tag="rec_bc")
                    nc.vector.reciprocal(rec_bc[:], ps_sums[:])

                    ps_attnP = attn_ps.tile([128, 512], f32, tag="attn")
                    with crit():
                        nc.tensor.matmul(
                            ps_attnP[0:64, :], wpr_sb[0:64, :], attn_sb[0:64, :],
                            start=True, stop=True, tile_position=(0, 0),
                            skip_group_check=True,
                        )
                        nc.tensor.matmul(
                            ps_attnP[64:128, :], wpr_sb[64:128, :], attn_sb[64:128, :],
                            start=True, stop=True, tile_position=(64, 64),
                            skip_group_check=True,
                        )
                    nc.vector.tensor_tensor(
                        cpr[blk][:], ps_attnP[:], rec_bc[:], OP.mult
                    )

                    # ---- c_proj for this s-block, interleaved with attention
                    for dt in range(8):
                        pcp = aux_ps.tile([128, 512], f32, tag="aux")
                        nc.tensor.matmul(
                            pcp[:], wcp_sb[:, dt * 128 : (dt + 1) * 128],
                            cpr[blk][:], start=True, stop=True,
                        )
                        ot = opool.tile([128, 512], f32, tag="ot")
                        if dt % 2 == 0:
                            nc.scalar.activation(
                                ot[:], pcp[:], AF.Identity,
                                bias=bcp_sb[:, dt : dt + 1],
                            )
                        else:
                            nc.vector.tensor_scalar(
                                ot[:], pcp[:], bcp_sb[:, dt : dt + 1], None, OP.add
                            )
                        nc.sync.dma_start(
                            out_d[dt][:, blk * 512 : (blk + 1) * 512], ot[:]
                        )

        for free in reversed(frees):
            free()


class TileCtx:
    """Thin helper so _build can use `tc.tile` / `tc.tile_pool` uniformly."""

    def __init__(self, tile_mod, nc):
        self._tc = tile_mod.TileContext(nc)

    def __enter__(self):
        self._tc.__enter__()
        return self._tc

    def __exit__(self, *exc):
        return self._tc.__exit__(*exc)


def _shard_inputs(inputs):
    import ml_dtypes

    bf = ml_dtypes.bfloat16
    # host-side input marshalling: transpose of hidden_states + bf16 rounding
    # (identical to the on-device SWDGE cast) for the matmul operands
    xT = np.ascontiguousarray(
        np.asarray(inputs["hidden_states"], dtype=np.float32).reshape(BS, D).T
    ).astype(bf)
    Wa = np.asarray(inputs["c_attn_w"], dtype=np.float32)
    ba = np.asarray(inputs["c_attn_b"], dtype=np.float32)
    Wp = np.asarray(inputs["c_proj_w"], dtype=np.float32)
    bp = np.asarray(inputs["c_proj_b"], dtype=np.float32)
    proj = np.asarray(inputs["projectors"], dtype=np.float32)

    in_maps = []
    F = HPC * HD
    for c in range(N_CORES):
        sl = slice(c * F, (c + 1) * F)
        in_maps.append(
            {
                "xT": xT,
                "w_qk": np.ascontiguousarray(
                    np.concatenate([Wa[:, sl], Wa[:, D + c * F : D + (c + 1) * F]], axis=1)
                ).astype(bf),
                "w_v": np.ascontiguousarray(
                    Wa[:, 2 * D + c * F : 2 * D + (c + 1) * F]
                ).astype(bf),
                "b_qk": np.ascontiguousarray(
                    np.concatenate([ba[sl], ba[D + c * F : D + (c + 1) * F]])
                ),
                "b_v": np.ascontiguousarray(ba[2 * D + c * F : 2 * D + (c + 1) * F]),
                "w_pr": np.ascontiguousarray(
                    proj[HPC * c : HPC * (c + 1)].reshape(F, HD)
                ).astype(bf),
                "w_cp": np.ascontiguousarray(Wp[sl, :]).astype(bf),
                "b_cp": bp if c == 0 else np.zeros_like(bp),
            }
        )
    return in_maps


def _get_nc():
    if "nc" not in _CACHE:
        from concourse import bacc

        nc = bacc.Bacc("TRN2", debug=False, num_devices=N_CORES)
        _build(nc)
        # Bacc.compile() runs generate_event_semaphores, which spills
        # per-instruction sync waits beyond the single HW wait slot into
        # separate EventSemaphore instructions — without it walrus fails
        # with "Too many sync wait commands".
        nc.compile()
        _CACHE["nc"] = nc
    return _CACHE["nc"]


def _run(inputs, trace=False, trace_kwargs=None):
    from concourse.bass_utils import run_bass_kernel_spmd

    nc = _get_nc()
    in_maps = _shard_inputs(inputs)
    res = run_bass_kernel_spmd(
        nc,
        in_maps,
        core_ids=list(range(N_CORES)),
        trace=trace,
        **(trace_kwargs or {}),
    )
    acc = np.zeros((8, 128, BS), dtype=np.float32)
    for r in res.results:
        acc += np.asarray(r["outT"], dtype=np.float32)
    out = acc.transpose(2, 0, 1).reshape(BS, D).reshape(B, S, D)
    return np.ascontiguousarray(out), res


def kernel(**inputs) -> np.ndarray:
    out, _ = _run(inputs, trace=False)
    return out


def simulate_core(inputs, core=0):
    """CoreSim one core's program (for correctness debugging). Returns outT."""
    from concourse.bass_interp import CoreSim

    nc = _get_nc()
    in_maps = _shard_inputs(inputs)
    sim = CoreSim(nc, trace=False)
    for name, arr in in_maps[core].items():
        sim.tensor(name)[:] = arr
    sim.simulate()
    return np.array(sim.tensor("outT"))



# revision 5
# speedup vs baseline: 1.2760x; 1.2760x over previous
# Trainium2 Bass kernel for nn_CompressedGPT2Attention.
#
# Model: B=2, S=2048, D=1024, H=16 heads of HD=64.
#   qkv = x @ c_attn_w + c_attn_b ; causal attention per head;
#   per-head symmetric projector on the attention output; out = attn @ c_proj_w + b.
#
# Sharding (megatron-style tensor parallel over heads, 8 cores x 2 heads):
#   - every core gets the full hidden_states
#   - c_attn (q,k,v) columns + projectors + c_proj rows are sharded by head
#   - each core computes a full-shape partial of the c_proj output; the
#     all-reduce after c_proj is done on the host (partials are summed there,
#     c_proj bias is also added on the host).
#
# On-core layout strategy: activations are kept feature-major ("transposed",
# features on SBUF partitions) so every matmul contracts over the partition
# dim without ever transposing big intermediates:
#   xT[d, s]   provided by the host (input marshalling), loaded sb-major so
#              compute starts after 1/8th of the load
#   qT,kT,vT[f,s] = W^T @ xT  (512-col streams; bias via per-partition
#              activation bias during the PSUM drain)
#   v_s[s, hd] = PE transpose of vT, 128x128 tiles (v must be seq-major as
#              the stationary operand of the attn matmul)
#   scoresT[kj, qi] = kT^T-slice matmuls (two heads packed on the PE via
#              tile_position row-tiling, K=64 each); diagonal blocks are
#              column-trimmed to the causally-valid region
#   expT = exp(scoresT/8) on ScalarE, causal mask via gpsimd affine_select
#   attn_unT[hd, qi] accumulated over kj with lhsT = v; softmax sums
#              ride along as a concurrent ones-column matmul; both are
#              software-pipelined one kj behind the score matmuls so the PE
#              never waits on ScalarE
#   normalization = reciprocal_approx_fast of the matmul-broadcast sums,
#              then one VectorE multiply
#   outT[dout, s] = c_proj partial with the per-head projector pre-folded
#              into the c_proj weights on the host (normalization is
#              per-(head,query) so it commutes with the projector); written
#              back fp32, host sums over cores. c_proj matmuls of block i-1
#              are injected mid-block i to hide the normalization latency.

import numpy as np

B, S, D, H, HD = 2, 2048, 1024, 16, 64
BS = B * S
N_CORES = 8
HPC = H // N_CORES  # heads per core = 2

_CACHE = {}


def _build(nc):
    import concourse.bass as bass
    import concourse.mybir as mybir
    import concourse.tile as tile
    from contextlib import ExitStack

    f32 = mybir.dt.float32
    bf16 = mybir.dt.bfloat16
    AF = mybir.ActivationFunctionType
    OP = mybir.AluOpType

    x_d = nc.dram_tensor("xT", [D, BS], bf16, kind="ExternalInput").ap()
    wqkv_d = nc.dram_tensor("w_qkv", [D, 3 * HPC * HD], bf16, kind="ExternalInput").ap()
    b3_d = nc.dram_tensor("b3", [HPC * HD, 3], f32, kind="ExternalInput").ap()
    wcp_d = nc.dram_tensor("w_cp", [HPC * HD, D], bf16, kind="ExternalInput").ap()
    ident_d = nc.dram_tensor("ident", [128, 128], bf16, kind="ExternalInput").ap()
    out_d = nc.dram_tensor("outT", [8, 128, BS], f32, kind="ExternalOutput").ap()

    F = HPC * HD  # 128 features per block (2 heads stacked)
    NB = BS // 512  # 8 s-blocks of 512
    KT = D // 128  # 8 contraction tiles

    with TileCtx(tile, nc) as tc:
        # ---------------- persistent tiles ----------------
        # tc.tile singles must be released in LIFO order, and their free
        # closures must be kept alive (GC of a discarded closure releases
        # the pool at a random trace point). xT goes last so it can be
        # freed right after the qkv phase.
        frees = []

        def ptile(shape, dtype, name):
            t, free = tc.tile(shape, dtype, name=name)
            frees.append(free)
            return t

        qT = ptile([128, BS], bf16, "qT")
        kTt = ptile([128, BS], bf16, "kTt")
        vT = ptile([128, BS], bf16, "vT")
        v_s = ptile([128, BS // 128, 128], bf16, "v_s")
        wqkv_sb = ptile([128, KT, 3 * F], bf16, "wqkv_sb")
        wcp_sb = ptile([128, D], bf16, "wcp_sb")
        b3_sb = ptile([128, 3], f32, "b3_sb")
        ones_w = ptile([128, 64], bf16, "ones_w")
        ident = ptile([128, 128], bf16, "ident")
        # one tile per 512-wide s-block so c_proj can start per-block
        cpr = [ptile([128, 512], bf16, f"cpr{i}") for i in range(NB)]

        xT, xT_free = tc.tile([128, KT, BS], bf16, name="xT")

        # ---------------- constants + weights ----------------
        nc.any.memset(ones_w[:], 1.0)
        nc.sync.dma_start(ident[:], ident_d)
        nc.sync.dma_start(wqkv_sb[:], wqkv_d.rearrange("(kt p) f -> p kt f", p=128))
        nc.sync.dma_start(b3_sb[:], b3_d)
        nc.sync.dma_start(wcp_sb[:], wcp_d)

        # ---------------- load x^T sb-major so sb=0 is ready early ---------
        for sb in range(NB):
            for kt in range(KT):
                nc.sync.dma_start(
                    xT[:, kt, sb * 512 : (sb + 1) * 512],
                    x_d[kt * 128 : (kt + 1) * 128, sb * 512 : (sb + 1) * 512],
                )

        # ---------------- q^T / k^T / v^T matmuls + v transpose -----------
        with ExitStack() as phase1:
            qkv_ps = phase1.enter_context(
                tc.tile_pool(name="qkv_ps", bufs=3, space="PSUM")
            )
            vt_ps = phase1.enter_context(
                tc.tile_pool(name="vt_ps", bufs=2, space="PSUM")
            )
            def transpose_sb(sb):
                # v_s[s, hd] tiles for this 512-col chunk via PE transpose
                # (gpsimd cannot read PSUM, so the drain lives on VectorE)
                ps_t = vt_ps.tile([128, 4, 128], bf16, tag="vt")
                for i in range(4):
                    st = sb * 4 + i
                    nc.tensor.transpose(
                        ps_t[:, i, :], vT[:, st * 128 : (st + 1) * 128], ident[:]
                    )
                nc.vector.tensor_copy(
                    v_s[:, sb * 4 : (sb + 1) * 4, :], ps_t[:]
                )

            for sb in range(NB):
                for ft, dest in ((0, qT), (1, kTt), (2, vT)):
                    ps = qkv_ps.tile([128, 512], f32, tag="qkv")
                    for kt in range(KT):
                        nc.tensor.matmul(
                            ps[:],
                            wqkv_sb[:, kt, ft * F : (ft + 1) * F],
                            xT[:, kt, sb * 512 : (sb + 1) * 512],
                            start=(kt == 0),
                            stop=(kt == KT - 1),
                        )
                    nc.scalar.activation(
                        dest[:, sb * 512 : (sb + 1) * 512], ps[:], AF.Identity,
                        bias=b3_sb[:, ft : ft + 1],
                    )
                # transposes lag one sb so the PE never waits on the vT drain
                if sb > 0:
                    transpose_sb(sb - 1)
            transpose_sb(NB - 1)
        xT_free()

        # ---------------- attention ----------------
        with ExitStack() as phase2:
            sc_ps = phase2.enter_context(tc.tile_pool(name="sc_ps", bufs=4, space="PSUM"))
            attn_ps = phase2.enter_context(tc.tile_pool(name="attn_ps", bufs=2, space="PSUM"))
            aux_ps = phase2.enter_context(tc.tile_pool(name="aux_ps", bufs=2, space="PSUM"))
            epool = phase2.enter_context(tc.tile_pool(name="epool", bufs=6))
            spool = phase2.enter_context(tc.tile_pool(name="spool", bufs=2))
            opool = phase2.enter_context(tc.tile_pool(name="opool", bufs=4))

            def emit_attn_sums(ps_attn, ps_sums, eA, eB, vs, c0, first, last):
                nc.tensor.matmul(
                    ps_attn[0:64, c0:512], vs[:, 0:64], eA[:, c0:512],
                    start=first, stop=last, tile_position=(0, 0),
                    skip_group_check=True,
                )
                nc.tensor.matmul(
                    ps_attn[64:128, c0:512], vs[:, 64:128], eB[:, c0:512],
                    start=first, stop=last, tile_position=(0, 64),
                    skip_group_check=True,
                )
                nc.tensor.matmul(
                    ps_sums[0:64, c0:512], ones_w[:, 0:64], eA[:, c0:512],
                    start=first, stop=last, tile_position=(0, 0),
                    skip_group_check=True,
                )
                nc.tensor.matmul(
                    ps_sums[64:128, c0:512], ones_w[:, 0:64], eB[:, c0:512],
                    start=first, stop=last, tile_position=(0, 64),
                    skip_group_check=True,
                )

            def emit_cproj(blk):
                for dt in range(8):
                    pcp = aux_ps.tile([128, 512], f32, tag="aux")
                    nc.tensor.matmul(
                        pcp[:], wcp_sb[:, dt * 128 : (dt + 1) * 128],
                        cpr[blk][:], start=True, stop=True,
                    )
                    ot = opool.tile([128, 512], f32, tag="ot")
                    nc.vector.tensor_copy(ot[:], pcp[:])
                    nc.sync.dma_start(
                        out_d[dt][:, blk * 512 : (blk + 1) * 512], ot[:]
                    )

            prev_blk = None
            for qt in range(4):
                for b in range(B):
                    blk = b * 4 + qt
                    qi = b * S + qt * 512
                    nkj = 4 * (qt + 1)
                    ps_attn = attn_ps.tile([128, 512], f32, tag="attn")
                    ps_sums = aux_ps.tile([128, 512], f32, tag="aux")
                    pending = None
                    for kj in range(nkj):
                        kjc = b * S + kj * 128
                        p = kj - 4 * qt
                        c0 = 128 * p if p > 0 else 0
                        pscA = sc_ps.tile([128, 512], f32, tag="sc")
                        pscB = sc_ps.tile([128, 512], f32, tag="sc")
                        nc.tensor.matmul(
                            pscA[:, c0:512], kTt[0:64, kjc : kjc + 128],
                            qT[0:64, qi + c0 : qi + 512],
                            start=True, stop=True, tile_position=(0, 0),
                        )
                        nc.tensor.matmul(
                            pscB[:, c0:512], kTt[64:128, kjc : kjc + 128],
                            qT[64:128, qi + c0 : qi + 512],
                            start=True, stop=True, tile_position=(64, 0),
                        )
                        # c_proj of the previous block, injected here so its
                        # normalization has finished by the time the PE
                        # reaches these matmuls
                        if kj == 1 and prev_blk is not None:
                            emit_cproj(prev_blk)
                            prev_blk = None
                        eA = epool.tile([128, 512], bf16, tag="e")
                        eB = epool.tile([128, 512], bf16, tag="e")
                        for e, psc in ((eA, pscA), (eB, pscB)):
                            nc.scalar.activation(
                                e[:, c0:512], psc[:, c0:512], AF.Exp, scale=0.125,
                            )
                            if p >= 0:
                                # triangle mask on the 128-wide diagonal square
                                nc.gpsimd.affine_select(
                                    e[:, 128 * p : 128 * (p + 1)],
                                    e[:, 128 * p : 128 * (p + 1)],
                                    pattern=[[1, 128]], base=0,
                                    channel_multiplier=-1,
                                    compare_op=OP.is_ge, fill=0.0,
                                )
                        if pending is not None:
                            emit_attn_sums(*pending)
                        vs = v_s[:, b * 16 + kj, :]
                        pending = (
                            ps_attn, ps_sums, eA, eB, vs, c0,
                            kj == 0, kj == nkj - 1,
                        )
                    emit_attn_sums(*pending)

                    # sums are matmul-broadcast across partitions, so one
                    # fast DVE reciprocal yields the normalization tile
                    rec_bc = spool.tile([128, 512], f32, tag="rec_bc")
                    nc.vector.reciprocal_approx_fast(rec_bc[:], ps_sums[:])
                    nc.vector.tensor_tensor(
                        cpr[blk][:], ps_attn[:], rec_bc[:], OP.mult
                    )
                    prev_blk = blk
            emit_cproj(prev_blk)

        for free in reversed(frees):
            free()


class TileCtx:
    """Thin helper so _build can use `tc.tile` / `tc.tile_pool` uniformly."""

    def __init__(self, tile_mod, nc):
        self._tc = tile_mod.TileContext(nc)

    def __enter__(self):
        self._tc.__enter__()
        return self._tc

    def __exit__(self, *exc):
        return self._tc.__exit__(*exc)


def _shard_inputs(inputs):
    import ml_dtypes

    bf = ml_dtypes.bfloat16
    # host-side input marshalling: transpose of hidden_states + bf16 rounding
    # (identical to the on-device SWDGE cast) for the matmul operands
    xT = np.ascontiguousarray(
        np.asarray(inputs["hidden_states"], dtype=np.float32).reshape(BS, D).T
    ).astype(bf)
    Wa = np.asarray(inputs["c_attn_w"], dtype=np.float32)
    ba = np.asarray(inputs["c_attn_b"], dtype=np.float32)
    Wp = np.asarray(inputs["c_proj_w"], dtype=np.float32)
    proj = np.asarray(inputs["projectors"], dtype=np.float32)
    ident = np.eye(128, dtype=np.float32).astype(bf)

    in_maps = []
    F = HPC * HD
    for c in range(N_CORES):
        sl = slice(c * F, (c + 1) * F)
        # fold the per-head projector into the c_proj rows for this core:
        # out_rows[h] = proj[h] @ Wcp[rows of head h]  (applied per head)
        wcp_fold = np.empty((F, D), dtype=np.float32)
        for j in range(HPC):
            h = HPC * c + j
            wcp_fold[j * HD : (j + 1) * HD] = (
                proj[h] @ Wp[c * F + j * HD : c * F + (j + 1) * HD, :]
            )
        b3 = np.stack(
            [
                ba[sl],
                ba[D + c * F : D + (c + 1) * F],
                ba[2 * D + c * F : 2 * D + (c + 1) * F],
            ],
            axis=1,
        )
        in_maps.append(
            {
                "xT": xT,
                "w_qkv": np.ascontiguousarray(
                    np.concatenate(
                        [
                            Wa[:, sl],
                            Wa[:, D + c * F : D + (c + 1) * F],
                            Wa[:, 2 * D + c * F : 2 * D + (c + 1) * F],
                        ],
                        axis=1,
                    )
                ).astype(bf),
                "b3": np.ascontiguousarray(b3),
                "w_cp": np.ascontiguousarray(wcp_fold).astype(bf),
                "ident": ident,
            }
        )
    return in_maps


def _get_nc():
    if "nc" not in _CACHE:
        from concourse import bacc

        nc = bacc.Bacc("TRN2", debug=False, num_devices=N_CORES)
        _build(nc)
        # Bacc.compile() runs generate_event_semaphores, which spills
        # per-instruction sync waits beyond the single HW wait slot into
        # separate EventSemaphore instructions — without it walrus fails
        # with "Too many sync wait commands".
        nc.compile()
        _CACHE["nc"] = nc
    return _CACHE["nc"]


def _run(inputs, trace=False, trace_kwargs=None):
    from concourse.bass_utils import run_bass_kernel_spmd

    nc = _get_nc()
    in_maps = _shard_inputs(inputs)
    res = run_bass_kernel_spmd(
        nc,
        in_maps,
        core_ids=list(range(N_CORES)),
        trace=trace,
        **(trace_kwargs or {}),
    )
    acc = np.zeros((8, 128, BS), dtype=np.float32)
    for r in res.results:
        acc += np.asarray(r["outT"], dtype=np.float32)
    out = acc.transpose(2, 0, 1).reshape(BS, D)
    out = out + np.asarray(inputs["c_proj_b"], dtype=np.float32)[None, :]
    out = out.reshape(B, S, D)
    return np.ascontiguousarray(out), res


def kernel(**inputs) -> np.ndarray:
    out, _ = _run(inputs, trace=False)
    return out


def simulate_core(inputs, core=0):
    """CoreSim one core's program (for correctness debugging). Returns outT."""
    from concourse.bass_interp import CoreSim

    nc = _get_nc()
    in_maps = _shard_inputs(inputs)
    sim = CoreSim(nc, trace=False)
    for name, arr in in_maps[core].items():
        sim.tensor(name)[:] = arr
    sim.simulate()
    return np.array(sim.tensor("outT"))


# revision 10
# speedup vs baseline: 1.3532x; 1.0605x over previous
# Trainium2 Bass kernel for nn_CompressedGPT2Attention.
#
# Model: B=2, S=2048, D=1024, H=16 heads of HD=64.
#   qkv = x @ c_attn_w + c_attn_b ; causal attention per head;
#   per-head symmetric projector on the attention output; out = attn @ c_proj_w + b.
#
# Sharding (megatron-style tensor parallel over heads, 8 cores x 2 heads):
#   - every core gets the full hidden_states
#   - c_attn (q,k,v) columns + projectors + c_proj rows are sharded by head
#   - each core computes a full-shape partial of the c_proj output; the
#     all-reduce after c_proj is done on the host (partials are summed there,
#     c_proj bias is also added on the host).
#
# On-core layout strategy: activations are kept feature-major ("transposed",
# features on SBUF partitions) so every matmul contracts over the partition
# dim without ever transposing big intermediates:
#   xT[d, s]   provided by the host (input marshalling), loaded sb-major so
#              compute starts after 1/8th of the load
#   qT,kT,vT[f,s] = W^T @ xT  (512-col streams; bias via per-partition
#              activation bias during the PSUM drain)
#   v_s[s, hd] = PE transpose of vT, 128x128 tiles (v must be seq-major as
#              the stationary operand of the attn matmul)
#   scoresT[kj, qi] = kT^T-slice matmuls (two heads packed on the PE via
#              tile_position row-tiling, K=64 each); diagonal blocks are
#              column-trimmed to the causally-valid region
#   expT = exp(scoresT/8) on ScalarE, causal mask via gpsimd affine_select
#   attn_unT[hd, qi] accumulated over kj with lhsT = v; softmax sums
#              ride along as a concurrent ones-column matmul; both are
#              software-pipelined one kj behind the score matmuls so the PE
#              never waits on ScalarE
#   normalization = reciprocal_approx_fast of the matmul-broadcast sums,
#              then one VectorE multiply
#   outT[dout, s] = c_proj partial with the per-head projector pre-folded
#              into the c_proj weights on the host (normalization is
#              per-(head,query) so it commutes with the projector); written
#              back fp32, host sums over cores. c_proj matmuls of block i-1
#              are injected mid-block i to hide the normalization latency.

import numpy as np

B, S, D, H, HD = 2, 2048, 1024, 16, 64
BS = B * S
N_CORES = 8
HPC = H // N_CORES  # heads per core = 2

_CACHE = {}


def _build(nc):
    import concourse.bass as bass
    import concourse.mybir as mybir
    import concourse.tile as tile
    from contextlib import ExitStack

    f32 = mybir.dt.float32
    bf16 = mybir.dt.bfloat16
    AF = mybir.ActivationFunctionType
    OP = mybir.AluOpType

    x_d = nc.dram_tensor("xT", [D, BS], bf16, kind="ExternalInput").ap()
    wqkv_d = nc.dram_tensor("w_qkv", [D, 3 * HPC * HD], bf16, kind="ExternalInput").ap()
    b3_d = nc.dram_tensor("b3", [HPC * HD, 3], f32, kind="ExternalInput").ap()
    wcp_d = nc.dram_tensor("w_cp", [HPC * HD, D], bf16, kind="ExternalInput").ap()
    ident_d = nc.dram_tensor("ident", [128, 128], bf16, kind="ExternalInput").ap()
    out_d = nc.dram_tensor("outT", [8, 128, BS], f32, kind="ExternalOutput").ap()

    F = HPC * HD  # 128 features per block (2 heads stacked)
    NB = BS // 512  # 8 s-blocks of 512
    KT = D // 128  # 8 contraction tiles

    with TileCtx(tile, nc) as tc:
        # ---------------- persistent tiles ----------------
        # tc.tile singles must be released in LIFO order, and their free
        # closures must be kept alive (GC of a discarded closure releases
        # the pool at a random trace point). xT goes last so it can be
        # freed right after the qkv phase.
        frees = []

        def ptile(shape, dtype, name):
            t, free = tc.tile(shape, dtype, name=name)
            frees.append(free)
            return t

        qT = ptile([128, BS], bf16, "qT")
        kTt = ptile([128, BS], bf16, "kTt")
        vT = ptile([128, BS], bf16, "vT")
        v_s = ptile([128, BS // 128, 128], bf16, "v_s")
        wqkv_sb = ptile([128, KT, 3 * F], bf16, "wqkv_sb")
        wcp_sb = ptile([128, D], bf16, "wcp_sb")
        b3_sb = ptile([128, 3], f32, "b3_sb")
        ones_w = ptile([128, 64], bf16, "ones_w")
        ident = ptile([128, 128], bf16, "ident")
        # one tile per 512-wide s-block so c_proj can start per-block
        cpr = [ptile([128, 512], bf16, f"cpr{i}") for i in range(NB)]

        xT, xT_free = tc.tile([128, KT, BS], bf16, name="xT")

        # ---------------- constants + weights ----------------
        # DMA issue order is tuned so the first q-matmul group (all kt of
        # sb=0) is unblocked as early as possible: per-kt wqkv row-blocks are
        # interleaved with sb=0's x chunks; wcp (first needed mid-phase-2)
        # goes after sb=1.
        nc.any.memset(ones_w[:], 1.0)
        nc.sync.dma_start(ident[:], ident_d)
        nc.sync.dma_start(b3_sb[:], b3_d)

        def load_x(sb, kt):
            nc.sync.dma_start(
                xT[:, kt, sb * 512 : (sb + 1) * 512],
                x_d[kt * 128 : (kt + 1) * 128, sb * 512 : (sb + 1) * 512],
            )

        for kt in range(KT):
            nc.sync.dma_start(
                wqkv_sb[:, kt, :], wqkv_d[kt * 128 : (kt + 1) * 128, :]
            )
            load_x(0, kt)
        for kt in range(KT):
            load_x(1, kt)
        nc.sync.dma_start(wcp_sb[:], wcp_d)
        for sb in range(2, NB):
            for kt in range(KT):
                load_x(sb, kt)

        # ---------------- q^T / k^T / v^T matmuls + v transpose -----------
        with ExitStack() as phase1:
            qkv_ps = phase1.enter_context(
                tc.tile_pool(name="qkv_ps", bufs=3, space="PSUM")
            )
            vt_ps = phase1.enter_context(
                tc.tile_pool(name="vt_ps", bufs=2, space="PSUM")
            )
            def transpose_sb(sb):
                # v_s[s, hd] tiles for this 512-col chunk via PE transpose
                # (gpsimd cannot read PSUM, so the drain lives on VectorE)
                ps_t = vt_ps.tile([128, 4, 128], bf16, tag="vt")
                for i in range(4):
                    st = sb * 4 + i
                    nc.tensor.transpose(
                        ps_t[:, i, :], vT[:, st * 128 : (st + 1) * 128], ident[:]
                    )
                nc.vector.tensor_copy(
                    v_s[:, sb * 4 : (sb + 1) * 4, :], ps_t[:]
                )

            for sb in range(NB):
                for ft, dest in ((0, qT), (1, kTt), (2, vT)):
                    ps = qkv_ps.tile([128, 512], f32, tag="qkv")
                    for kt in range(KT):
                        nc.tensor.matmul(
                            ps[:],
                            wqkv_sb[:, kt, ft * F : (ft + 1) * F],
                            xT[:, kt, sb * 512 : (sb + 1) * 512],
                            start=(kt == 0),
                            stop=(kt == KT - 1),
                        )
                    nc.scalar.activation(
                        dest[:, sb * 512 : (sb + 1) * 512], ps[:], AF.Identity,
                        bias=b3_sb[:, ft : ft + 1],
                    )
                # transposes lag one sb so the PE never waits on the vT drain
                if sb > 0:
                    transpose_sb(sb - 1)
            transpose_sb(NB - 1)
        xT_free()

        # ---------------- attention ----------------
        with ExitStack() as phase2:
            sc_ps = phase2.enter_context(tc.tile_pool(name="sc_ps", bufs=2, space="PSUM"))
            attn_ps = phase2.enter_context(tc.tile_pool(name="attn_ps", bufs=2, space="PSUM"))
            aux_ps = phase2.enter_context(tc.tile_pool(name="aux_ps", bufs=2, space="PSUM"))
            epool = phase2.enter_context(tc.tile_pool(name="epool", bufs=3))
            spool = phase2.enter_context(tc.tile_pool(name="spool", bufs=2))
            opool = phase2.enter_context(tc.tile_pool(name="opool", bufs=4))

            def emit_attn_sums(ps_attn, ps_sums, eAB, vs, c0, first, last):
                nc.tensor.matmul(
                    ps_attn[0:64, c0:512], vs[:, 0:64], eAB[:, 0, c0:512],
                    start=first, stop=last, tile_position=(0, 0),
                    skip_group_check=True,
                )
                nc.tensor.matmul(
                    ps_attn[64:128, c0:512], vs[:, 64:128], eAB[:, 1, c0:512],
                    start=first, stop=last, tile_position=(0, 64),
                    skip_group_check=True,
                )
                nc.tensor.matmul(
                    ps_sums[0:64, c0:512], ones_w[:, 0:64], eAB[:, 0, c0:512],
                    start=first, stop=last, tile_position=(0, 0),
                    skip_group_check=True,
                )
                nc.tensor.matmul(
                    ps_sums[64:128, c0:512], ones_w[:, 0:64], eAB[:, 1, c0:512],
                    start=first, stop=last, tile_position=(0, 64),
                    skip_group_check=True,
                )

            def emit_cproj(blk):
                for dt in range(8):
                    pcp = aux_ps.tile([128, 512], f32, tag="aux")
                    nc.tensor.matmul(
                        pcp[:], wcp_sb[:, dt * 128 : (dt + 1) * 128],
                        cpr[blk][:], start=True, stop=True,
                    )
                    ot = opool.tile([128, 512], f32, tag="ot")
                    nc.vector.tensor_copy(ot[:], pcp[:])
                    nc.sync.dma_start(
                        out_d[dt][:, blk * 512 : (blk + 1) * 512], ot[:]
                    )

            prev_blk = None
            for qt in range(4):
                for b in range(B):
                    blk = b * 4 + qt
                    qi = b * S + qt * 512
                    nkj = 4 * (qt + 1)
                    ps_attn = attn_ps.tile([128, 512], f32, tag="attn")
                    ps_sums = aux_ps.tile([128, 512], f32, tag="aux")
                    pending = None
                    for kj in range(nkj):
                        kjc = b * S + kj * 128
                        p = kj - 4 * qt
                        c0 = 128 * p if p > 0 else 0
                        # both heads' scores go into one 2-bank psum tile so
                        # the exp pair is a single ScalarE instruction
                        pscAB = sc_ps.tile([128, 2, 512], f32, tag="sc")
                        nc.tensor.matmul(
                            pscAB[:, 0, c0:512], kTt[0:64, kjc : kjc + 128],
                            qT[0:64, qi + c0 : qi + 512],
                            start=True, stop=True, tile_position=(0, 0),
                        )
                        nc.tensor.matmul(
                            pscAB[:, 1, c0:512], kTt[64:128, kjc : kjc + 128],
                            qT[64:128, qi + c0 : qi + 512],
                            start=True, stop=True, tile_position=(64, 0),
                        )
                        # c_proj of the previous block, injected here so its
                        # normalization has finished by the time the PE
                        # reaches these matmuls
                        if kj == 1 and prev_blk is not None:
                            emit_cproj(prev_blk)
                            prev_blk = None
                        eAB = epool.tile([128, 2, 512], bf16, tag="e")
                        nc.scalar.activation(
                            eAB[:, :, c0:512], pscAB[:, :, c0:512],
                            AF.Exp, scale=0.125,
                        )
                        if p >= 0:
                            # triangle mask on the 128-wide diagonal square
                            for h in range(2):
                                nc.gpsimd.affine_select(
                                    eAB[:, h, 128 * p : 128 * (p + 1)],
                                    eAB[:, h, 128 * p : 128 * (p + 1)],
                                    pattern=[[1, 128]], base=0,
                                    channel_multiplier=-1,
                                    compare_op=OP.is_ge, fill=0.0,
                                )
                        if pending is not None:
                            emit_attn_sums(*pending)
                        vs = v_s[:, b * 16 + kj, :]
                        pending = (
                            ps_attn, ps_sums, eAB, vs, c0,
                            kj == 0, kj == nkj - 1,
                        )
                    emit_attn_sums(*pending)

                    # sums are matmul-broadcast across partitions, so one
                    # fast DVE reciprocal yields the normalization tile
                    rec_bc = spool.tile([128, 512], f32, tag="rec_bc")
                    nc.vector.reciprocal_approx_fast(rec_bc[:], ps_sums[:])
                    nc.vector.tensor_tensor(
                        cpr[blk][:], ps_attn[:], rec_bc[:], OP.mult
                    )
                    prev_blk = blk
            emit_cproj(prev_blk)

        for free in reversed(frees):
            free()


class TileCtx:
    """Thin helper so _build can use `tc.tile` / `tc.tile_pool` uniformly."""

    def __init__(self, tile_mod, nc):
        self._tc = tile_mod.TileContext(nc)

    def __enter__(self):
        self._tc.__enter__()
        return self._tc

    def __exit__(self, *exc):
        return self._tc.__exit__(*exc)


def _shard_inputs(inputs):
    import ml_dtypes

    bf = ml_dtypes.bfloat16
    # host-side input marshalling: transpose of hidden_states + bf16 rounding
    # (identical to the on-device SWDGE cast) for the matmul operands
    xT = np.ascontiguousarray(
        np.asarray(inputs["hidden_states"], dtype=np.float32).reshape(BS, D).T
    ).astype(bf)
    Wa = np.asarray(inputs["c_attn_w"], dtype=np.float32)
    ba = np.asarray(inputs["c_attn_b"], dtype=np.float32)
    Wp = np.asarray(inputs["c_proj_w"], dtype=np.float32)
    proj = np.asarray(inputs["projectors"], dtype=np.float32)
    ident = np.eye(128, dtype=np.float32).astype(bf)

    in_maps = []
    F = HPC * HD
    for c in range(N_CORES):
        sl = slice(c * F, (c + 1) * F)
        # fold the per-head projector into the c_proj rows for this core:
        # out_rows[h] = proj[h] @ Wcp[rows of head h]  (applied per head)
        wcp_fold = np.empty((F, D), dtype=np.float32)
        for j in range(HPC):
            h = HPC * c + j
            wcp_fold[j * HD : (j + 1) * HD] = (
                proj[h] @ Wp[c * F + j * HD : c * F + (j + 1) * HD, :]
            )
        b3 = np.stack(
            [
                ba[sl],
                ba[D + c * F : D + (c + 1) * F],
                ba[2 * D + c * F : 2 * D + (c + 1) * F],
            ],
            axis=1,
        )
        in_maps.append(
            {
                "xT": xT,
                "w_qkv": np.ascontiguousarray(
                    np.concatenate(
                        [
                            Wa[:, sl],
                            Wa[:, D + c * F : D + (c + 1) * F],
                            Wa[:, 2 * D + c * F : 2 * D + (c + 1) * F],
                        ],
                        axis=1,
                    )
                ).astype(bf),
                "b3": np.ascontiguousarray(b3),
                "w_cp": np.ascontiguousarray(wcp_fold).astype(bf),
                "ident": ident,
            }
        )
    return in_maps


def _get_nc():
    if "nc" not in _CACHE:
        from concourse import bacc

        nc = bacc.Bacc("TRN2", debug=False, num_devices=N_CORES)
        _build(nc)
        # Bacc.compile() runs generate_event_semaphores, which spills
        # per-instruction sync waits beyond the single HW wait slot into
        # separate EventSemaphore instructions — without it walrus fails
        # with "Too many sync wait commands".
        nc.compile()
        _CACHE["nc"] = nc
    return _CACHE["nc"]


def _run(inputs, trace=False, trace_kwargs=None):
    from concourse.bass_utils import run_bass_kernel_spmd

    nc = _get_nc()
    in_maps = _shard_inputs(inputs)
    res = run_bass_kernel_spmd(
        nc,
        in_maps,
        core_ids=list(range(N_CORES)),
        trace=trace,
        **(trace_kwargs or {}),
    )
    acc = np.zeros((8, 128, BS), dtype=np.float32)
    for r in res.results:
        acc += np.asarray(r["outT"], dtype=np.float32)
    out = acc.transpose(2, 0, 1).reshape(BS, D)
    out = out + np.asarray(inputs["c_proj_b"], dtype=np.float32)[None, :]
    out = out.reshape(B, S, D)
    return np.ascontiguousarray(out), res


def kernel(**inputs) -> np.ndarray:
    out, _ = _run(inputs, trace=False)
    return out


def simulate_core(inputs, core=0):
    """CoreSim one core's program (for correctness debugging). Returns outT."""
    from concourse.bass_interp import CoreSim

    nc = _get_nc()
    in_maps = _shard_inputs(inputs)
    sim = CoreSim(nc, trace=False)
    for name, arr in in_maps[core].items():
        sim.tensor(name)[:] = arr
    sim.simulate()
    return np.array(sim.tensor("outT"))


# revision 14
# speedup vs baseline: 1.4708x; 1.0869x over previous
# Trainium2 Bass kernel for nn_CompressedGPT2Attention.
#
# Model: B=2, S=2048, D=1024, H=16 heads of HD=64.
#   qkv = x @ c_attn_w + c_attn_b ; causal attention per head;
#   per-head symmetric projector on the attention output; out = attn @ c_proj_w + b.
#
# Sharding (megatron-style tensor parallel over heads, 8 cores x 2 heads):
#   - every core gets the full hidden_states
#   - c_attn (q,k,v) columns + projectors + c_proj rows are sharded by head
#   - each core computes a full-shape partial of the c_proj output; the
#     all-reduce after c_proj is done on the host (partials are summed there,
#     c_proj bias is also added on the host).
#
# On-core layout strategy: activations are kept feature-major ("transposed",
# features on SBUF partitions) so every matmul contracts over the partition
# dim without ever transposing big intermediates:
#   xT[d, s]   provided by the host (input marshalling), loaded sb-major so
#              compute starts after 1/8th of the load
#   qT,kT,vT[f,s] = W^T @ xT  (512-col streams; bias via per-partition
#              activation bias during the PSUM drain)
#   v_s[s, hd] = PE transpose of vT, 128x128 tiles (v must be seq-major as
#              the stationary operand of the attn matmul)
#   scoresT[kj, qi] = kT^T-slice matmuls (two heads packed on the PE via
#              tile_position row-tiling, K=64 each); diagonal blocks are
#              column-trimmed to the causally-valid region
#   expT = exp(scoresT/8) on ScalarE, causal mask via gpsimd affine_select
#   attn_unT[hd, qi] accumulated over kj with lhsT = v; softmax sums
#              ride along as a concurrent ones-column matmul; both are
#              software-pipelined one kj behind the score matmuls so the PE
#              never waits on ScalarE
#   normalization = reciprocal_approx_fast of the matmul-broadcast sums,
#              then one VectorE multiply
#   outT[dout, s] = c_proj partial with the per-head projector pre-folded
#              into the c_proj weights on the host (normalization is
#              per-(head,query) so it commutes with the projector); written
#              back fp32, host sums over cores. c_proj matmuls of block i-1
#              are injected mid-block i to hide the normalization latency.

import numpy as np

B, S, D, H, HD = 2, 2048, 1024, 16, 64
BS = B * S
N_CORES = 8
HPC = H // N_CORES  # heads per core = 2

_CACHE = {}


def _build(nc):
    import concourse.bass as bass
    import concourse.mybir as mybir
    import concourse.tile as tile
    from contextlib import ExitStack

    f32 = mybir.dt.float32
    bf16 = mybir.dt.bfloat16
    AF = mybir.ActivationFunctionType
    OP = mybir.AluOpType

    x_d = nc.dram_tensor("xT", [D, BS], bf16, kind="ExternalInput").ap()
    wqkv_d = nc.dram_tensor("w_qkv", [D, 3 * HPC * HD], bf16, kind="ExternalInput").ap()
    b3_d = nc.dram_tensor("b3", [HPC * HD, 3], f32, kind="ExternalInput").ap()
    wcp_d = nc.dram_tensor("w_cp", [HPC * HD, D], bf16, kind="ExternalInput").ap()
    ident_d = nc.dram_tensor("ident", [128, 128], bf16, kind="ExternalInput").ap()
    out_d = nc.dram_tensor("outT", [8, 128, BS], f32, kind="ExternalOutput").ap()

    F = HPC * HD  # 128 features per block (2 heads stacked)
    NB = BS // 512  # 8 s-blocks of 512
    KT = D // 128  # 8 contraction tiles

    with TileCtx(tile, nc) as tc:
        # ---------------- persistent tiles ----------------
        # tc.tile singles must be released in LIFO order, and their free
        # closures must be kept alive (GC of a discarded closure releases
        # the pool at a random trace point). xT goes last so it can be
        # freed right after the qkv phase.
        frees = []

        def ptile(shape, dtype, name):
            t, free = tc.tile(shape, dtype, name=name)
            frees.append(free)
            return t

        qT = ptile([128, BS], bf16, "qT")
        kTt = ptile([128, BS], bf16, "kTt")
        vT = ptile([128, BS], bf16, "vT")
        v_s = ptile([128, BS // 128, 128], bf16, "v_s")
        wqkv_sb = ptile([128, KT, 3 * F], bf16, "wqkv_sb")
        wcp_sb = ptile([128, D], bf16, "wcp_sb")
        b3_sb = ptile([128, 3], f32, "b3_sb")
        ones_w = ptile([128, 64], bf16, "ones_w")
        ident = ptile([128, 128], bf16, "ident")
        # one tile per 512-wide s-block so c_proj can start per-block
        cpr = [ptile([128, 512], bf16, f"cpr{i}") for i in range(NB)]

        xT, xT_free = tc.tile([128, KT, BS], bf16, name="xT")

        # ---------------- constants + weights ----------------
        # DMA issue order is tuned so the first q-matmul group (all kt of
        # sb=0) is unblocked as early as possible: per-kt wqkv row-blocks are
        # interleaved with sb=0's x chunks; wcp (first needed mid-phase-2)
        # goes after sb=1.
        nc.any.memset(ones_w[:], 1.0)
        nc.sync.dma_start(ident[:], ident_d)
        nc.sync.dma_start(b3_sb[:], b3_d)

        def load_x(sb, kt):
            nc.sync.dma_start(
                xT[:, kt, sb * 512 : (sb + 1) * 512],
                x_d[kt * 128 : (kt + 1) * 128, sb * 512 : (sb + 1) * 512],
            )

        for kt in range(KT):
            nc.sync.dma_start(
                wqkv_sb[:, kt, :], wqkv_d[kt * 128 : (kt + 1) * 128, :]
            )
            load_x(0, kt)
        for kt in range(KT):
            load_x(1, kt)
        nc.sync.dma_start(wcp_sb[:], wcp_d)
        for sb in range(2, NB):
            for kt in range(KT):
                load_x(sb, kt)

        # ---------------- q^T / k^T / v^T matmuls + v transpose -----------
        with ExitStack() as phase1:
            qkv_ps = phase1.enter_context(
                tc.tile_pool(name="qkv_ps", bufs=3, space="PSUM")
            )
            vt_ps = phase1.enter_context(
                tc.tile_pool(name="vt_ps", bufs=2, space="PSUM")
            )
            def transpose_sb(sb):
                # v_s[s, hd] tiles for this 512-col chunk via PE transpose
                # (gpsimd cannot read PSUM, so the drain lives on VectorE)
                ps_t = vt_ps.tile([128, 4, 128], bf16, tag="vt")
                for i in range(4):
                    st = sb * 4 + i
                    nc.tensor.transpose(
                        ps_t[:, i, :], vT[:, st * 128 : (st + 1) * 128], ident[:]
                    )
                nc.vector.tensor_copy(
                    v_s[:, sb * 4 : (sb + 1) * 4, :], ps_t[:]
                )

            for sb in range(NB):
                for ft, dest in ((0, qT), (1, kTt), (2, vT)):
                    ps = qkv_ps.tile([128, 512], f32, tag="qkv")
                    for kt in range(KT):
                        nc.tensor.matmul(
                            ps[:],
                            wqkv_sb[:, kt, ft * F : (ft + 1) * F],
                            xT[:, kt, sb * 512 : (sb + 1) * 512],
                            start=(kt == 0),
                            stop=(kt == KT - 1),
                        )
                    nc.scalar.activation(
                        dest[:, sb * 512 : (sb + 1) * 512], ps[:], AF.Identity,
                        bias=b3_sb[:, ft : ft + 1],
                    )
                # transposes lag one sb so the PE never waits on the vT drain
                if sb > 0:
                    transpose_sb(sb - 1)
            transpose_sb(NB - 1)
        xT_free()

        # ---------------- attention ----------------
        with ExitStack() as phase2:
            sc_ps = phase2.enter_context(tc.tile_pool(name="sc_ps", bufs=2, space="PSUM"))
            attn_ps = phase2.enter_context(tc.tile_pool(name="attn_ps", bufs=2, space="PSUM"))
            aux_ps = phase2.enter_context(tc.tile_pool(name="aux_ps", bufs=2, space="PSUM"))
            epool = phase2.enter_context(tc.tile_pool(name="epool", bufs=3))
            spool = phase2.enter_context(tc.tile_pool(name="spool", bufs=2))
            opool = phase2.enter_context(tc.tile_pool(name="opool", bufs=4))

            def emit_attn_sums(ps_attn, ps_sums, eAB, vs, c0, first, last):
                nc.tensor.matmul(
                    ps_attn[0:64, c0:512], vs[:, 0:64], eAB[:, 0, c0:512],
                    start=first, stop=last, tile_position=(0, 0),
                    skip_group_check=True,
                )
                nc.tensor.matmul(
                    ps_attn[64:128, c0:512], vs[:, 64:128], eAB[:, 1, c0:512],
                    start=first, stop=last, tile_position=(0, 64),
                    skip_group_check=True,
                )
                nc.tensor.matmul(
                    ps_sums[0:64, c0:512], ones_w[:, 0:64], eAB[:, 0, c0:512],
                    start=first, stop=last, tile_position=(0, 0),
                    skip_group_check=True,
                )
                nc.tensor.matmul(
                    ps_sums[64:128, c0:512], ones_w[:, 0:64], eAB[:, 1, c0:512],
                    start=first, stop=last, tile_position=(0, 64),
                    skip_group_check=True,
                )

            odrain = [nc.vector, nc.scalar]

            def emit_cproj(blk):
                # drains alternate VectorE/ScalarE so the 2-buf psum rotation
                # is gated by matmul pace, not a single engine's copy chain
                for dt in range(8):
                    pcp = aux_ps.tile([128, 512], f32, tag="aux")
                    nc.tensor.matmul(
                        pcp[:], wcp_sb[:, dt * 128 : (dt + 1) * 128],
                        cpr[blk][:], start=True, stop=True,
                    )
                    ot = opool.tile([128, 512], f32, tag="ot")
                    if dt % 2 == 0:
                        nc.vector.tensor_copy(ot[:], pcp[:])
                    else:
                        nc.scalar.copy(ot[:], pcp[:])
                    nc.sync.dma_start(
                        out_d[dt][:, blk * 512 : (blk + 1) * 512], ot[:]
                    )

            prev_blk = None
            for qt in range(4):
                for b in range(B):
                    blk = b * 4 + qt
                    qi = b * S + qt * 512
                    nkj = 4 * (qt + 1)
                    ps_attn = attn_ps.tile([128, 512], f32, tag="attn")
                    ps_sums = aux_ps.tile([128, 512], f32, tag="aux")
                    pending = None
                    for kj in range(nkj):
                        kjc = b * S + kj * 128
                        p = kj - 4 * qt
                        c0 = 128 * p if p > 0 else 0
                        # both heads' scores go into one 2-bank psum tile so
                        # the exp pair is a single ScalarE instruction
                        pscAB = sc_ps.tile([128, 2, 512], f32, tag="sc")
                        nc.tensor.matmul(
                            pscAB[:, 0, c0:512], kTt[0:64, kjc : kjc + 128],
                            qT[0:64, qi + c0 : qi + 512],
                            start=True, stop=True, tile_position=(0, 0),
                        )
                        nc.tensor.matmul(
                            pscAB[:, 1, c0:512], kTt[64:128, kjc : kjc + 128],
                            qT[64:128, qi + c0 : qi + 512],
                            start=True, stop=True, tile_position=(64, 0),
                        )
                        # c_proj of the previous block, injected here so its
                        # normalization has finished by the time the PE
                        # reaches these matmuls
                        if kj == 2 and prev_blk is not None:
                            emit_cproj(prev_blk)
                            prev_blk = None
                        eAB = epool.tile([128, 2, 512], bf16, tag="e")
                        nc.scalar.activation(
                            eAB[:, :, c0:512], pscAB[:, :, c0:512],
                            AF.Exp, scale=0.125,
                        )
                        if p >= 0:
                            # triangle mask on the 128-wide diagonal square
                            for h in range(2):
                                nc.gpsimd.affine_select(
                                    eAB[:, h, 128 * p : 128 * (p + 1)],
                                    eAB[:, h, 128 * p : 128 * (p + 1)],
                                    pattern=[[1, 128]], base=0,
                                    channel_multiplier=-1,
                                    compare_op=OP.is_ge, fill=0.0,
                                )
                        if pending is not None:
                            emit_attn_sums(*pending)
                        vs = v_s[:, b * 16 + kj, :]
                        pending = (
                            ps_attn, ps_sums, eAB, vs, c0,
                            kj == 0, kj == nkj - 1,
                        )
                    emit_attn_sums(*pending)

                    # sums are matmul-broadcast across partitions, so one
                    # fast DVE reciprocal yields the normalization tile
                    # (both tensor_tensor inputs can't be PSUM, so the
                    # reciprocal doubles as the PSUM->SBUF move)
                    rec_bc = spool.tile([128, 512], f32, tag="rec_bc")
                    nc.vector.reciprocal_approx_fast(rec_bc[:], ps_sums[:])
                    nc.vector.tensor_tensor(
                        cpr[blk][:], ps_attn[:], rec_bc[:], OP.mult
                    )
                    prev_blk = blk
            emit_cproj(prev_blk)

        for free in reversed(frees):
            free()


class TileCtx:
    """Thin helper so _build can use `tc.tile` / `tc.tile_pool` uniformly."""

    def __init__(self, tile_mod, nc):
        self._tc = tile_mod.TileContext(nc)

    def __enter__(self):
        self._tc.__enter__()
        return self._tc

    def __exit__(self, *exc):
        return self._tc.__exit__(*exc)


def _shard_inputs(inputs):
    import ml_dtypes

    bf = ml_dtypes.bfloat16
    # host-side input marshalling: transpose of hidden_states + bf16 rounding
    # (identical to the on-device SWDGE cast) for the matmul operands
    xT = np.ascontiguousarray(
        np.asarray(inputs["hidden_states"], dtype=np.float32).reshape(BS, D).T
    ).astype(bf)
    Wa = np.asarray(inputs["c_attn_w"], dtype=np.float32)
    ba = np.asarray(inputs["c_attn_b"], dtype=np.float32)
    Wp = np.asarray(inputs["c_proj_w"], dtype=np.float32)
    proj = np.asarray(inputs["projectors"], dtype=np.float32)
    ident = np.eye(128, dtype=np.float32).astype(bf)

    in_maps = []
    F = HPC * HD
    for c in range(N_CORES):
        sl = slice(c * F, (c + 1) * F)
        # fold the per-head projector into the c_proj rows for this core:
        # out_rows[h] = proj[h] @ Wcp[rows of head h]  (applied per head)
        wcp_fold = np.empty((F, D), dtype=np.float32)
        for j in range(HPC):
            h = HPC * c + j
            wcp_fold[j * HD : (j + 1) * HD] = (
                proj[h] @ Wp[c * F + j * HD : c * F + (j + 1) * HD, :]
            )
        b3 = np.stack(
            [
                ba[sl],
                ba[D + c * F : D + (c + 1) * F],
                ba[2 * D + c * F : 2 * D + (c + 1) * F],
            ],
            axis=1,
        )
        in_maps.append(
            {
                "xT": xT,
                "w_qkv": np.ascontiguousarray(
                    np.concatenate(
                        [
                            Wa[:, sl],
                            Wa[:, D + c * F : D + (c + 1) * F],
                            Wa[:, 2 * D + c * F : 2 * D + (c + 1) * F],
                        ],
                        axis=1,
                    )
                ).astype(bf),
                "b3": np.ascontiguousarray(b3),
                "w_cp": np.ascontiguousarray(wcp_fold).astype(bf),
                "ident": ident,
            }
        )
    return in_maps


def _get_nc():
    if "nc" not in _CACHE:
        from concourse import bacc

        nc = bacc.Bacc("TRN2", debug=False, num_devices=N_CORES)
        _build(nc)
        # Bacc.compile() runs generate_event_semaphores, which spills
        # per-instruction sync waits beyond the single HW wait slot into
        # separate EventSemaphore instructions — without it walrus fails
        # with "Too many sync wait commands".
        nc.compile()
        _CACHE["nc"] = nc
    return _CACHE["nc"]


def _run(inputs, trace=False, trace_kwargs=None):
    from concourse.bass_utils import run_bass_kernel_spmd

    nc = _get_nc()
    in_maps = _shard_inputs(inputs)
    res = run_bass_kernel_spmd(
        nc,
        in_maps,
        core_ids=list(range(N_CORES)),
        trace=trace,
        **(trace_kwargs or {}),
    )
    acc = np.zeros((8, 128, BS), dtype=np.float32)
    for r in res.results:
        acc += np.asarray(r["outT"], dtype=np.float32)
    out = acc.transpose(2, 0, 1).reshape(BS, D)
    out = out + np.asarray(inputs["c_proj_b"], dtype=np.float32)[None, :]
    out = out.reshape(B, S, D)
    return np.ascontiguousarray(out), res


def kernel(**inputs) -> np.ndarray:
    out, _ = _run(inputs, trace=False)
    return out


def simulate_core(inputs, core=0):
    """CoreSim one core's program (for correctness debugging). Returns outT."""
    from concourse.bass_interp import CoreSim

    nc = _get_nc()
    in_maps = _shard_inputs(inputs)
    sim = CoreSim(nc, trace=False)
    for name, arr in in_maps[core].items():
        sim.tensor(name)[:] = arr
    sim.simulate()
    return np.array(sim.tensor("outT"))
